# revision 37
# baseline (speedup 1.0000x reference)
"""Bass/Trainium2 kernel for DynamicMultiheadAttention (sparse_attention).

Sharding: 8 cores = (batch b in {0,1}) x (query-slice of 512 rows).
Each core computes all 8 heads for its (b, n-slice) in transposed
orientation: scores sT[m, n] with keys m on partitions, so that
  - the relative-mask bias  -sum_r c[h,r]*M_r[m,n]  is accumulated into
    score PSUM by matmuls with a scaled-identity stationary operand,
  - softmax row-sums come free from a ones-column appended to V,
  - attn @ V needs no transposes (pT tiles are directly the stationary
    operand layout).
Key padding is applied by zeroing padded key rows of V and of the
ones-column (exactly equivalent to -inf logits). The row-constant term
scale_h * sum_r w[h,r] = scale_h cancels in softmax and is dropped; the
k-projection bias is softmax-invariant and dropped; the v bias folds
into the output bias (softmax rows sum to 1): bo' = bv @ Wo + bo.

Every TPB instruction encoding in this walrus build tolerates only ONE
semaphore wait. Two mechanisms deal with that: tiny "touch" matmuls
absorb the DMA-queue waits for PE-consumed loads (one wait each), and a
post-pass (_split_matmul_waits) moves any remaining extra waits onto
standalone single-wait EventSemaphore instructions inserted before the
offending instruction on the same engine queue.

All matmuls run as float32r (1 PE cycle/row vs 4 for fp32); projection
inputs x/W are bf16 (halves the load bytes; the score/mask/attention
path stays f32r). Measured end-to-end Frobenius relative error vs the
fp32 reference: ~5e-3.
"""

import numpy as np

N, B, D = 2048, 2, 512
H, R = 8, 3
C = D // H          # 64
NS = N // 4         # 512 query rows per core
NCORES = 8
MT = N // 128       # 16 key tiles

_cache = {}


def _build_program(reps=1):
    import concourse.bass as bass
    import concourse.mybir as mybir
    import concourse.tile as tile
    from concourse.tile import add_dep_helper
    from contextlib import ExitStack

    f32 = mybir.dt.float32
    f32r = mybir.dt.float32r
    bf16 = mybir.dt.bfloat16
    u8 = mybir.dt.uint8
    AFT = mybir.ActivationFunctionType
    ALU = mybir.AluOpType

    nc = bass.Bass()

    xtq = nc.declare_dram_parameter("xtq", [D, NS], bf16, isOutput=False)
    xtk = nc.declare_dram_parameter("xtk", [D, N], bf16, isOutput=False)
    xtv = nc.declare_dram_parameter("xtv", [D, N], bf16, isOutput=False)
    masksT = nc.declare_dram_parameter("masksT", [R, N, NS], u8, isOutput=False)
    wq = nc.declare_dram_parameter("wq", [D, D], bf16, isOutput=False)
    wk = nc.declare_dram_parameter("wk", [D, D], bf16, isOutput=False)
    wv = nc.declare_dram_parameter("wv", [D, D], bf16, isOutput=False)
    wo = nc.declare_dram_parameter("wo", [D, D], bf16, isOutput=False)
    idents = nc.declare_dram_parameter("idents", [H * R, 128, 128], f32r, isOutput=False)
    bq2 = nc.declare_dram_parameter("bq2", [128, 4], f32, isOutput=False)
    bo2 = nc.declare_dram_parameter("bo2", [128, 4], f32, isOutput=False)
    pad = nc.declare_dram_parameter("pad", [128, MT], f32, isOutput=False)
    pad8 = nc.declare_dram_parameter("pad8", [128, MT, H], f32, isOutput=False)
    outT = nc.declare_dram_parameter("outT", [D, NS], f32, isOutput=True)

    with tile.TileContext(nc) as tc, ExitStack() as ctx:
        # fp32 matmul is 4 cycles/row on PE; fp32r streams at 1 --
        # all matmul-operand tiles are float32r
        mm = nc.tensor.matmul

        for _rep in range(reps):
            _run_once(nc, tc, ctx, mm, tile, mybir, f32, f32r, bf16, u8,
                      AFT, ALU, xtq, xtk, xtv, masksT, wq, wk, wv, wo,
                      idents, bq2, bo2, pad, pad8, outT)

    _split_matmul_waits(nc, mybir)
    return nc


def _run_once(nc, tc, ctx, mm, tile, mybir, f32, f32r, bf16, u8, AFT, ALU,
              xtq, xtk, xtv, masksT, wq, wk, wv, wo, idents, bq2, bo2,
              pad, pad8, outT):
    from concourse.tile import add_dep_helper
    from contextlib import ExitStack
    with ExitStack() as ctx:
        const_pool = ctx.enter_context(tc.tile_pool(name="const", bufs=1))
        persist = ctx.enter_context(tc.tile_pool(name="persist", bufs=1))

        loads = []
        id_sb = const_pool.tile([128, H * R, 128], f32r)
        nc.sync.dma_start(id_sb[:], idents.rearrange("i p m -> p i m"))
        bq_sb = const_pool.tile([128, 4], f32)
        loads.append(nc.sync.dma_start(bq_sb[:], bq2[:]))
        bo_sb = const_pool.tile([128, 4], f32)
        loads.append(nc.sync.dma_start(bo_sb[:], bo2[:]))
        pad_sb = const_pool.tile([128, MT], f32)
        loads.append(nc.sync.dma_start(pad_sb[:], pad[:]))
        pad8_sb = const_pool.tile([128, MT, H], f32)
        loads.append(nc.sync.dma_start(pad8_sb[:], pad8[:]))
        ones_sb = const_pool.tile([1, 64], f32)
        loads.append(nc.vector.memset(ones_sb[:], 1.0))
        wo_sb = persist.tile([128, 4, D], bf16)
        for c in range(4):
            loads.append(nc.sync.dma_start(wo_sb[:, c, :], wo[c * 128:(c + 1) * 128, :]))

        kT_sb = persist.tile([128, 4, N], f32r)
        qT_sb = persist.tile([128, 4, NS], f32r)
        v_sb = persist.tile([128, MT, H, C + 1], f32r)
        OT_sb = persist.tile([128, 4, NS], bf16)
        outT_sb = persist.tile([128, 4, NS], f32)

        # ---- Phase A: projections ----
        with tc.tile_pool(name="xw", bufs=1) as xw_pool, \
             tc.tile_pool(name="psA", bufs=3, space="PSUM") as psA:
            wq_sb = xw_pool.tile([128, 4, D], bf16, tag="w")
            wk_sb = xw_pool.tile([128, 4, D], bf16, tag="w2")
            wv_sb = xw_pool.tile([128, 4, D], bf16, tag="w3")
            xtq_sb = xw_pool.tile([128, 4, NS], bf16, tag="xq")
            xtk_sb = xw_pool.tile([128, 4, N], bf16, tag="xk")
            xtv_sb = xw_pool.tile([128, 4, N], bf16, tag="xv")
            for c in range(4):
                sl = slice(c * 128, (c + 1) * 128)
                loads.append(nc.sync.dma_start(wq_sb[:, c, :], wq[sl, :]))
                loads.append(nc.sync.dma_start(xtq_sb[:, c, :], xtq[sl, :]))
            for c in range(4):
                sl = slice(c * 128, (c + 1) * 128)
                loads.append(nc.sync.dma_start(wk_sb[:, c, :], wk[sl, :]))
                loads.append(nc.sync.dma_start(xtk_sb[:, c, :], xtk[sl, :]))
            for c in range(4):
                sl = slice(c * 128, (c + 1) * 128)
                loads.append(nc.sync.dma_start(wv_sb[:, c, :], wv[sl, :]))
                loads.append(nc.sync.dma_start(xtv_sb[:, c, :], xtv[sl, :]))

            vones = []
            for mt in range(MT):
                vones.append(nc.vector.tensor_copy(
                    v_sb[:, mt, :, C : C + 1],
                    pad8_sb[:, mt, :].rearrange("p (h o) -> p h o", o=1)))

            # tiny touch-matmuls absorb DMA-queue waits (Matmult tolerates
            # only one semaphore wait) -- one touch per DMA whose data is
            # consumed by PE, touching exactly that DMA's region
            with tc.tile_pool(name="psD", bufs=1, space="PSUM") as psD:
                dummy_ps = psD.tile([1, 2], f32)
                for c in range(4):
                    for t in (xtq_sb, xtk_sb, xtv_sb, wo_sb):
                        a = t[0:1, c, 0:1]
                        nc.tensor.matmul(dummy_ps[0:1, 0:1], a, a,
                                         start=True, stop=True)
                a = id_sb[0:1, 0, 0:1].bitcast(f32)
                nc.tensor.matmul(dummy_ps[0:1, 0:1], a, a,
                                 start=True, stop=True)

            projc = []
            # qT[dh, n] = (Wq/8).T @ xT_q  (+ bq/8 per-partition)
            for j in range(4):
                ps = psA.tile([128, NS], f32, tag="psA")
                for kc in range(4):
                    mm(ps[:], wq_sb[:, kc, j * 128:(j + 1) * 128],
                       xtq_sb[:, kc, :], start=(kc == 0), stop=(kc == 3))
                projc.append(nc.scalar.activation(
                    qT_sb[:, j, :], ps[:], AFT.Identity,
                    bias=bq_sb[:, j:j + 1]))

            # kT[dh, m] = Wk.T @ xT_k   (k bias is softmax-invariant: dropped)
            for mb in range(4):
                for j in range(4):
                    ps = psA.tile([128, NS], f32, tag="psA")
                    for kc in range(4):
                        mm(ps[:], wk_sb[:, kc, j * 128:(j + 1) * 128],
                           xtk_sb[:, kc, mb * 512:(mb + 1) * 512],
                           start=(kc == 0), stop=(kc == 3))
                    if j < 2:
                        projc.append(nc.scalar.copy(
                            kT_sb[:, j, mb * 512:(mb + 1) * 512], ps[:]))
                    else:
                        projc.append(nc.vector.tensor_copy(
                            kT_sb[:, j, mb * 512:(mb + 1) * 512], ps[:]))

            # v[m, c] = xT_v.T @ Wv, padded key rows zeroed (scale by pad01)
            for mt in range(MT):
                ps = psA.tile([128, D], f32, tag="psA")
                for kc in range(4):
                    mm(ps[:], xtv_sb[:, kc, mt * 128:(mt + 1) * 128],
                       wv_sb[:, kc, :], start=(kc == 0), stop=(kc == 3))
                projc.append(nc.vector.tensor_scalar(
                    v_sb[:, mt, :, 0:C],
                    ps[:].rearrange("p (h c) -> p h c", h=H),
                    pad_sb[:, mt:mt + 1], None, ALU.mult))

        # ---- Phase B: attention, two passes of 4 heads ----
        with tc.tile_pool(name="mu8", bufs=2) as mu8_pool, \
             tc.tile_pool(name="mc", bufs=2) as mc_pool, \
             tc.tile_pool(name="pT", bufs=3) as pT_pool, \
             tc.tile_pool(name="small", bufs=8) as small_pool, \
             tc.tile_pool(name="psO", bufs=4, space="PSUM") as psO, \
             tc.tile_pool(name="psB", bufs=2, space="PSUM") as psB, \
             tc.tile_pool(name="psS", bufs=2, space="PSUM") as psS:
            prev_norm = []
            for p in range(2):
                o_ps = [psO.tile([128, NS], f32, tag="psO", name=f"o_ps{p}_{i}")
                        for i in range(4)]
                for mt in range(MT):
                    m_u8 = mu8_pool.tile([128, R, NS], u8, tag="mu8")
                    for r in range(R):
                        nc.sync.dma_start(m_u8[:, r, :],
                                          masksT[r, mt * 128:(mt + 1) * 128, :])
                    mc = mc_pool.tile([128, R, NS], f32r, tag="mc")
                    cast = nc.vector.tensor_copy(mc[:], m_u8[:])
                    for pn in prev_norm:
                        add_dep_helper(cast.ins, pn.ins, sync=False,
                                       reason="order cast after prior norm")
                    prev_norm = []
                    for i in range(4):
                        h = 4 * p + i
                        hj, ho = h // 2, (h % 2) * 64
                        s_ps = psS.tile([128, NS], f32, tag="psS")
                        mm(s_ps[:],
                           kT_sb[ho:ho + 64, hj, mt * 128:(mt + 1) * 128],
                           qT_sb[ho:ho + 64, hj, :], start=True, stop=False)
                        for r in range(R):
                            mm(s_ps[:], id_sb[:, h * R + r, :],
                               mc[:, r, :], start=False, stop=(r == 2))
                        pT = pT_pool.tile([128, NS], f32r, tag="pT")
                        nc.scalar.activation(pT[:], s_ps[:], AFT.Exp)
                        mm(o_ps[i][0:65, :], v_sb[:, mt, h, :], pT[:],
                           start=(mt == 0), stop=(mt == MT - 1))
                # normalize: OT[h-rows, n] = o[c, n] / rowsum[n]
                norm = []
                for i in range(4):
                    h = 4 * p + i
                    hj, ho = h // 2, (h % 2) * 64
                    rsb = small_pool.tile([1, NS], f32, tag="rsb")
                    nc.vector.reciprocal(rsb[:], o_ps[i][64:65, :])
                    # bcast matmul + psum consumers kept DVE-only so the
                    # matmul's deps collapse to one semaphore wait
                    b_ps = psB.tile([128, NS], f32, tag="psB")
                    mm(b_ps[0:64, :], ones_sb[0:1, :], rsb[0:1, :],
                       start=True, stop=True)
                    b_sb = small_pool.tile([64, NS], f32, tag="bsb")
                    nc.vector.tensor_copy(b_sb[:], b_ps[0:64, :])
                    norm.append(nc.vector.tensor_tensor(
                        OT_sb[ho:ho + 64, hj, :], o_ps[i][0:64, :],
                        b_sb[:], ALU.mult))
                prev_norm = norm

            # ---- Phase C: output projection ----
            for jt in range(4):
                ps = psS.tile([128, NS], f32, tag="psS")
                for g in range(4):
                    mm(ps[:], wo_sb[:, g, jt * 128:(jt + 1) * 128],
                       OT_sb[:, g, :], start=(g == 0), stop=(g == 3))
                nc.scalar.activation(outT_sb[:, jt, :], ps[:], AFT.Identity,
                                     bias=bo_sb[:, jt:jt + 1])
                nc.sync.dma_start(outT[jt * 128:(jt + 1) * 128, :],
                                  outT_sb[:, jt, :])


# every TPB instruction encoding in this walrus build tolerates only a
# single semaphore wait -- split extras regardless of opcode
_NO_SPLIT_TYPES = {"InstEventSemaphore"}


def _split_matmul_waits(nc, mybir):
    """Several engine instruction encodings tolerate only one semaphore
    wait; move extra waits onto standalone single-wait EventSemaphore
    instructions inserted right before them on the same engine queue."""
    import bass_rust

    n = 0
    for bb in nc.m.functions[0].blocks:
        insts = list(bb.instructions)
        out = []
        changed = False
        for i in insts:
            si = i.sync_info
            if (type(i).__name__ not in _NO_SPLIT_TYPES and si is not None
                    and len(si.on_wait) > 1):
                w = list(si.on_wait)
                for wx in w[:-1]:
                    ev = mybir.InstEventSemaphore(name=f"mmw_{n}_{i.name}",
                                                  ins=[], outs=[])
                    ev.engine = i.engine
                    ev.sync_info = bass_rust.SyncInfo(on_wait=[wx],
                                                      on_update=[])
                    out.append(ev)
                    n += 1
                si.on_wait = [w[-1]]
                changed = True
            out.append(i)
        if changed:
            bb.instructions = out


def _host_prep(inputs):
    x_q = np.asarray(inputs["x_q"], np.float32)
    x_k = np.asarray(inputs["x_k"], np.float32)
    x_v = np.asarray(inputs["x_v"], np.float32)
    attn_mask = np.asarray(inputs["attn_mask"]).astype(np.uint8)
    kpm = np.asarray(inputs["key_padding_mask"]).astype(bool)
    Wq = np.asarray(inputs["Wq"], np.float32)
    Wk = np.asarray(inputs["Wk"], np.float32)
    Wv = np.asarray(inputs["Wv"], np.float32)
    Wo = np.asarray(inputs["Wo"], np.float32)
    bq = np.asarray(inputs["bq"], np.float32)
    bv = np.asarray(inputs["bv"], np.float32)
    bo = np.asarray(inputs["bo"], np.float32)
    mw = np.asarray(inputs["mask_weight"], np.float64)

    # c[h,r] = softmax(mask_weight[h,:R]) * mask_weight[h,R]
    e = np.exp(mw[:, :R] - mw[:, :R].max(axis=1, keepdims=True))
    w = e / e.sum(axis=1, keepdims=True)
    c = (w * mw[:, R:R + 1]).astype(np.float32)          # [H, R]

    idents = np.zeros((H * R, 128, 128), np.float32)
    eye = np.eye(128, dtype=np.float32)
    for h in range(H):
        for r in range(R):
            idents[h * R + r] = eye * (-c[h, r])

    scale = np.float32(1.0 / np.sqrt(C))
    wq_s = (Wq * scale).astype(np.float32)
    bq_s = (bq * scale).astype(np.float32)
    bo_p = (bv @ Wo + bo).astype(np.float32)

    bq2 = np.ascontiguousarray(bq_s.reshape(4, 128).T)
    bo2 = np.ascontiguousarray(bo_p.reshape(4, 128).T)

    bf = ml_dtypes.bfloat16
    common = dict(wq=wq_s.astype(bf), wk=Wk.astype(bf), wv=Wv.astype(bf),
                  wo=Wo.astype(bf), idents=idents, bq2=bq2, bo2=bo2)

    in_maps = []
    for core in range(NCORES):
        b, ns = core // 4, core % 4
        n0 = ns * NS
        pad01 = (~kpm[b]).astype(np.float32)             # [N]
        pad2 = np.ascontiguousarray(pad01.reshape(MT, 128).T)
        pad8 = np.ascontiguousarray(np.repeat(pad2[:, :, None], H, axis=2))
        m = dict(common)
        m["xtq"] = np.ascontiguousarray(x_q[n0:n0 + NS, b, :].T).astype(bf)
        m["xtk"] = np.ascontiguousarray(x_k[:, b, :].T).astype(bf)
        m["xtv"] = np.ascontiguousarray(x_v[:, b, :].T).astype(bf)
        m["masksT"] = np.ascontiguousarray(
            attn_mask[b, :, n0:n0 + NS, :].transpose(0, 2, 1))
        m["pad"] = pad2
        m["pad8"] = pad8
        in_maps.append(m)
    return in_maps


def kernel(**inputs) -> np.ndarray:
    from concourse.bass_utils import run_bass_kernel_spmd

    if "nc" not in _cache:
        _cache["nc"] = _build_program()
    nc = _cache["nc"]

    in_maps = _host_prep(inputs)
    res = run_bass_kernel_spmd(nc, in_maps, list(range(NCORES)))

    out = np.empty((N, B, D), np.float32)
    for core in range(NCORES):
        b, ns = core // 4, core % 4
        n0 = ns * NS
        out[n0:n0 + NS, b, :] = res.results[core]["outT"].T
    return out


# revision 42
# speedup vs baseline: 1.0560x; 1.0560x over previous
"""Bass/Trainium2 kernel for DynamicMultiheadAttention (sparse_attention).

Sharding: 8 cores = (batch b in {0,1}) x (query-slice of 512 rows).
Each core computes all 8 heads for its (b, n-slice) in transposed
orientation: scores sT[m, n] with keys m on partitions, so that
  - the relative-mask bias  -sum_r c[h,r]*M_r[m,n]  is accumulated into
    score PSUM by matmuls with a scaled-identity stationary operand,
  - softmax row-sums come free from a ones-column appended to V,
  - attn @ V needs no transposes (pT tiles are directly the stationary
    operand layout).
Key padding is applied by zeroing padded key rows of V and of the
ones-column (exactly equivalent to -inf logits). The row-constant term
scale_h * sum_r w[h,r] = scale_h cancels in softmax and is dropped; the
k-projection bias is softmax-invariant and dropped; the v bias folds
into the output bias (softmax rows sum to 1): bo' = bv @ Wo + bo.

Every TPB instruction encoding in this walrus build tolerates only ONE
semaphore wait. Two mechanisms deal with that: tiny "touch" matmuls
absorb the DMA-queue waits for PE-consumed loads (one wait each), and a
post-pass (_split_matmul_waits) moves any remaining extra waits onto
standalone single-wait EventSemaphore instructions inserted before the
offending instruction on the same engine queue.

All matmuls run as float32r (1 PE cycle/row vs 4 for fp32); projection
inputs x/W are bf16 (halves the load bytes; the score/mask/attention
path stays f32r). Measured end-to-end Frobenius relative error vs the
fp32 reference: ~5e-3.
"""

import numpy as np

N, B, D = 2048, 2, 512
H, R = 8, 3
C = D // H          # 64
NS = N // 4         # 512 query rows per core
NCORES = 8
MT = N // 128       # 16 key tiles

_cache = {}


def _build_program(reps=1):
    import concourse.bass as bass
    import concourse.mybir as mybir
    import concourse.tile as tile
    from concourse.tile import add_dep_helper
    from contextlib import ExitStack

    f32 = mybir.dt.float32
    f32r = mybir.dt.float32r
    bf16 = mybir.dt.bfloat16
    u8 = mybir.dt.uint8
    AFT = mybir.ActivationFunctionType
    ALU = mybir.AluOpType

    nc = bass.Bass()

    xtq = nc.declare_dram_parameter("xtq", [D, NS], bf16, isOutput=False)
    xtk = nc.declare_dram_parameter("xtk", [D, N], bf16, isOutput=False)
    xtv = nc.declare_dram_parameter("xtv", [D, N], bf16, isOutput=False)
    masksT = nc.declare_dram_parameter("masksT", [R, N, NS], u8, isOutput=False)
    wq = nc.declare_dram_parameter("wq", [D, D], bf16, isOutput=False)
    wk = nc.declare_dram_parameter("wk", [D, D], bf16, isOutput=False)
    wv = nc.declare_dram_parameter("wv", [D, D], bf16, isOutput=False)
    wo = nc.declare_dram_parameter("wo", [D, D], bf16, isOutput=False)
    idents = nc.declare_dram_parameter("idents", [H * R, 128, 128], f32r, isOutput=False)
    bq2 = nc.declare_dram_parameter("bq2", [128, 4], f32, isOutput=False)
    bo2 = nc.declare_dram_parameter("bo2", [128, 4], f32, isOutput=False)
    pad = nc.declare_dram_parameter("pad", [128, MT], f32, isOutput=False)
    pad8 = nc.declare_dram_parameter("pad8", [128, MT, H], f32, isOutput=False)
    outT = nc.declare_dram_parameter("outT", [D, NS], f32, isOutput=True)

    with tile.TileContext(nc) as tc, ExitStack() as ctx:
        # fp32 matmul is 4 cycles/row on PE; fp32r streams at 1 --
        # all matmul-operand tiles are float32r
        mm = nc.tensor.matmul

        for _rep in range(reps):
            _run_once(nc, tc, ctx, mm, tile, mybir, f32, f32r, bf16, u8,
                      AFT, ALU, xtq, xtk, xtv, masksT, wq, wk, wv, wo,
                      idents, bq2, bo2, pad, pad8, outT)

    _split_matmul_waits(nc, mybir)
    return nc


def _run_once(nc, tc, ctx, mm, tile, mybir, f32, f32r, bf16, u8, AFT, ALU,
              xtq, xtk, xtv, masksT, wq, wk, wv, wo, idents, bq2, bo2,
              pad, pad8, outT):
    from concourse.tile import add_dep_helper
    from contextlib import ExitStack
    with ExitStack() as ctx:
        const_pool = ctx.enter_context(tc.tile_pool(name="const", bufs=1))
        persist = ctx.enter_context(tc.tile_pool(name="persist", bufs=1))

        loads = []
        id_sb = const_pool.tile([128, H * R, 128], f32r)
        nc.sync.dma_start(id_sb[:], idents.rearrange("i p m -> p i m"))
        bq_sb = const_pool.tile([128, 4], f32)
        loads.append(nc.sync.dma_start(bq_sb[:], bq2[:]))
        bo_sb = const_pool.tile([128, 4], f32)
        loads.append(nc.sync.dma_start(bo_sb[:], bo2[:]))
        pad_sb = const_pool.tile([128, MT], f32)
        loads.append(nc.sync.dma_start(pad_sb[:], pad[:]))
        pad8_sb = const_pool.tile([128, MT, H], f32)
        loads.append(nc.sync.dma_start(pad8_sb[:], pad8[:]))
        ones_sb = const_pool.tile([1, 64], f32)
        loads.append(nc.vector.memset(ones_sb[:], 1.0))
        wo_sb = persist.tile([128, 4, D], bf16)
        for c in range(4):
            loads.append(nc.sync.dma_start(wo_sb[:, c, :], wo[c * 128:(c + 1) * 128, :]))

        kT_sb = persist.tile([128, 4, N], f32r)
        qT_sb = persist.tile([128, 4, NS], f32r)
        v_sb = persist.tile([128, MT, H, C + 1], f32r)
        OT_sb = persist.tile([128, 4, NS], bf16)
        outT_sb = persist.tile([128, 4, NS], f32)

        # ---- Phase A: projections ----
        with tc.tile_pool(name="xw", bufs=1) as xw_pool, \
             tc.tile_pool(name="psA", bufs=3, space="PSUM") as psA:
            wq_sb = xw_pool.tile([128, 4, D], bf16, tag="w")
            wk_sb = xw_pool.tile([128, 4, D], bf16, tag="w2")
            wv_sb = xw_pool.tile([128, 4, D], bf16, tag="w3")
            xtq_sb = xw_pool.tile([128, 4, NS], bf16, tag="xq")
            xtk_sb = xw_pool.tile([128, 4, N], bf16, tag="xk")
            xtv_sb = xw_pool.tile([128, 4, N], bf16, tag="xv")
            for c in range(4):
                sl = slice(c * 128, (c + 1) * 128)
                loads.append(nc.sync.dma_start(wq_sb[:, c, :], wq[sl, :]))
                loads.append(nc.sync.dma_start(xtq_sb[:, c, :], xtq[sl, :]))
            for c in range(4):
                sl = slice(c * 128, (c + 1) * 128)
                loads.append(nc.sync.dma_start(wk_sb[:, c, :], wk[sl, :]))
                loads.append(nc.sync.dma_start(xtk_sb[:, c, :], xtk[sl, :]))
            for c in range(4):
                sl = slice(c * 128, (c + 1) * 128)
                loads.append(nc.sync.dma_start(wv_sb[:, c, :], wv[sl, :]))
                loads.append(nc.sync.dma_start(xtv_sb[:, c, :], xtv[sl, :]))

            vones = []
            for mt in range(MT):
                vones.append(nc.vector.tensor_copy(
                    v_sb[:, mt, :, C : C + 1],
                    pad8_sb[:, mt, :].rearrange("p (h o) -> p h o", o=1)))

            projc = []
            # qT[dh, n] = (Wq/8).T @ xT_q  (+ bq/8 per-partition)
            for j in range(4):
                ps = psA.tile([128, NS], f32, tag="psA")
                for kc in range(4):
                    mm(ps[:], wq_sb[:, kc, j * 128:(j + 1) * 128],
                       xtq_sb[:, kc, :], start=(kc == 0), stop=(kc == 3))
                projc.append(nc.scalar.activation(
                    qT_sb[:, j, :], ps[:], AFT.Identity,
                    bias=bq_sb[:, j:j + 1]))

            # kT[dh, m] = Wk.T @ xT_k   (k bias is softmax-invariant: dropped)
            for mb in range(4):
                for j in range(4):
                    ps = psA.tile([128, NS], f32, tag="psA")
                    for kc in range(4):
                        mm(ps[:], wk_sb[:, kc, j * 128:(j + 1) * 128],
                           xtk_sb[:, kc, mb * 512:(mb + 1) * 512],
                           start=(kc == 0), stop=(kc == 3))
                    if j < 2:
                        projc.append(nc.scalar.copy(
                            kT_sb[:, j, mb * 512:(mb + 1) * 512], ps[:]))
                    else:
                        projc.append(nc.vector.tensor_copy(
                            kT_sb[:, j, mb * 512:(mb + 1) * 512], ps[:]))

            # v[m, c] = xT_v.T @ Wv, padded key rows zeroed (scale by pad01)
            for mt in range(MT):
                ps = psA.tile([128, D], f32, tag="psA")
                for kc in range(4):
                    mm(ps[:], xtv_sb[:, kc, mt * 128:(mt + 1) * 128],
                       wv_sb[:, kc, :], start=(kc == 0), stop=(kc == 3))
                projc.append(nc.vector.tensor_scalar(
                    v_sb[:, mt, :, 0:C],
                    ps[:].rearrange("p (h c) -> p h c", h=H),
                    pad_sb[:, mt:mt + 1], None, ALU.mult))

        # ---- Phase B: attention, two passes of 4 heads ----
        with tc.tile_pool(name="mu8", bufs=2) as mu8_pool, \
             tc.tile_pool(name="mc", bufs=2) as mc_pool, \
             tc.tile_pool(name="pT", bufs=3) as pT_pool, \
             tc.tile_pool(name="small", bufs=8) as small_pool, \
             tc.tile_pool(name="psO", bufs=4, space="PSUM") as psO, \
             tc.tile_pool(name="psB", bufs=2, space="PSUM") as psB, \
             tc.tile_pool(name="psS", bufs=2, space="PSUM") as psS:
            # masks fit in SBUF -- DMA once (pass 0), reuse in pass 1
            mall = persist.tile([128, MT, R, NS], u8, name="mall")
            prev_norm = []
            for p in range(2):
                o_ps = [psO.tile([128, NS], f32, tag="psO", name=f"o_ps{p}_{i}")
                        for i in range(4)]
                for mt in range(MT):
                    if p == 0:
                        for r in range(R):
                            nc.sync.dma_start(
                                mall[:, mt, r, :],
                                masksT[r, mt * 128:(mt + 1) * 128, :])
                    mc = mc_pool.tile([128, R, NS], f32r, tag="mc")
                    cast = nc.vector.tensor_copy(mc[:], mall[:, mt, :, :])
                    for pn in prev_norm:
                        add_dep_helper(cast.ins, pn.ins, sync=False,
                                       reason="order cast after prior norm")
                    prev_norm = []
                    for i in range(4):
                        h = 4 * p + i
                        hj, ho = h // 2, (h % 2) * 64
                        s_ps = psS.tile([128, NS], f32, tag="psS")
                        mm(s_ps[:],
                           kT_sb[ho:ho + 64, hj, mt * 128:(mt + 1) * 128],
                           qT_sb[ho:ho + 64, hj, :], start=True, stop=False)
                        for r in range(R):
                            mm(s_ps[:], id_sb[:, h * R + r, :],
                               mc[:, r, :], start=False, stop=(r == 2))
                        pT = pT_pool.tile([128, NS], f32r, tag="pT")
                        nc.scalar.activation(pT[:], s_ps[:], AFT.Exp)
                        mm(o_ps[i][0:65, :], v_sb[:, mt, h, :], pT[:],
                           start=(mt == 0), stop=(mt == MT - 1))
                # normalize: OT[h-rows, n] = o[c, n] / rowsum[n]
                norm = []
                for i in range(4):
                    h = 4 * p + i
                    hj, ho = h // 2, (h % 2) * 64
                    rsb = small_pool.tile([1, NS], f32, tag="rsb")
                    nc.vector.reciprocal(rsb[:], o_ps[i][64:65, :])
                    # bcast matmul + psum consumers kept DVE-only so the
                    # matmul's deps collapse to one semaphore wait
                    b_ps = psB.tile([128, NS], f32, tag="psB")
                    mm(b_ps[0:64, :], ones_sb[0:1, :], rsb[0:1, :],
                       start=True, stop=True)
                    b_sb = small_pool.tile([64, NS], f32, tag="bsb")
                    nc.vector.tensor_copy(b_sb[:], b_ps[0:64, :])
                    norm.append(nc.vector.tensor_tensor(
                        OT_sb[ho:ho + 64, hj, :], o_ps[i][0:64, :],
                        b_sb[:], ALU.mult))
                prev_norm = norm

            # ---- Phase C: output projection ----
            for jt in range(4):
                ps = psS.tile([128, NS], f32, tag="psS")
                for g in range(4):
                    mm(ps[:], wo_sb[:, g, jt * 128:(jt + 1) * 128],
                       OT_sb[:, g, :], start=(g == 0), stop=(g == 3))
                nc.scalar.activation(outT_sb[:, jt, :], ps[:], AFT.Identity,
                                     bias=bo_sb[:, jt:jt + 1])
                nc.sync.dma_start(outT[jt * 128:(jt + 1) * 128, :],
                                  outT_sb[:, jt, :])


# every TPB instruction encoding in this walrus build tolerates only a
# single semaphore wait -- split extras regardless of opcode
_NO_SPLIT_TYPES = {"InstEventSemaphore"}


def _split_matmul_waits(nc, mybir):
    """Several engine instruction encodings tolerate only one semaphore
    wait; move extra waits onto standalone single-wait EventSemaphore
    instructions inserted right before them on the same engine queue."""
    import bass_rust

    n = 0
    for bb in nc.m.functions[0].blocks:
        insts = list(bb.instructions)
        out = []
        changed = False
        for i in insts:
            si = i.sync_info
            if (type(i).__name__ not in _NO_SPLIT_TYPES and si is not None
                    and len(si.on_wait) > 1):
                w = list(si.on_wait)
                for wx in w[:-1]:
                    ev = mybir.InstEventSemaphore(name=f"mmw_{n}_{i.name}",
                                                  ins=[], outs=[])
                    ev.engine = i.engine
                    ev.sync_info = bass_rust.SyncInfo(on_wait=[wx],
                                                      on_update=[])
                    out.append(ev)
                    n += 1
                si.on_wait = [w[-1]]
                changed = True
            out.append(i)
        if changed:
            bb.instructions = out


def _host_prep(inputs):
    x_q = np.asarray(inputs["x_q"], np.float32)
    x_k = np.asarray(inputs["x_k"], np.float32)
    x_v = np.asarray(inputs["x_v"], np.float32)
    attn_mask = np.asarray(inputs["attn_mask"]).astype(np.uint8)
    kpm = np.asarray(inputs["key_padding_mask"]).astype(bool)
    Wq = np.asarray(inputs["Wq"], np.float32)
    Wk = np.asarray(inputs["Wk"], np.float32)
    Wv = np.asarray(inputs["Wv"], np.float32)
    Wo = np.asarray(inputs["Wo"], np.float32)
    bq = np.asarray(inputs["bq"], np.float32)
    bv = np.asarray(inputs["bv"], np.float32)
    bo = np.asarray(inputs["bo"], np.float32)
    mw = np.asarray(inputs["mask_weight"], np.float64)

    # c[h,r] = softmax(mask_weight[h,:R]) * mask_weight[h,R]
    e = np.exp(mw[:, :R] - mw[:, :R].max(axis=1, keepdims=True))
    w = e / e.sum(axis=1, keepdims=True)
    c = (w * mw[:, R:R + 1]).astype(np.float32)          # [H, R]

    idents = np.zeros((H * R, 128, 128), np.float32)
    eye = np.eye(128, dtype=np.float32)
    for h in range(H):
        for r in range(R):
            idents[h * R + r] = eye * (-c[h, r])

    scale = np.float32(1.0 / np.sqrt(C))
    wq_s = (Wq * scale).astype(np.float32)
    bq_s = (bq * scale).astype(np.float32)
    bo_p = (bv @ Wo + bo).astype(np.float32)

    bq2 = np.ascontiguousarray(bq_s.reshape(4, 128).T)
    bo2 = np.ascontiguousarray(bo_p.reshape(4, 128).T)

    bf = ml_dtypes.bfloat16
    common = dict(wq=wq_s.astype(bf), wk=Wk.astype(bf), wv=Wv.astype(bf),
                  wo=Wo.astype(bf), idents=idents, bq2=bq2, bo2=bo2)

    in_maps = []
    for core in range(NCORES):
        b, ns = core // 4, core % 4
        n0 = ns * NS
        pad01 = (~kpm[b]).astype(np.float32)             # [N]
        pad2 = np.ascontiguousarray(pad01.reshape(MT, 128).T)
        pad8 = np.ascontiguousarray(np.repeat(pad2[:, :, None], H, axis=2))
        m = dict(common)
        m["xtq"] = np.ascontiguousarray(x_q[n0:n0 + NS, b, :].T).astype(bf)
        m["xtk"] = np.ascontiguousarray(x_k[:, b, :].T).astype(bf)
        m["xtv"] = np.ascontiguousarray(x_v[:, b, :].T).astype(bf)
        m["masksT"] = np.ascontiguousarray(
            attn_mask[b, :, n0:n0 + NS, :].transpose(0, 2, 1))
        m["pad"] = pad2
        m["pad8"] = pad8
        in_maps.append(m)
    return in_maps


def kernel(**inputs) -> np.ndarray:
    from concourse.bass_utils import run_bass_kernel_spmd

    if "nc" not in _cache:
        _cache["nc"] = _build_program()
    nc = _cache["nc"]

    in_maps = _host_prep(inputs)
    res = run_bass_kernel_spmd(nc, in_maps, list(range(NCORES)))

    out = np.empty((N, B, D), np.float32)
    for core in range(NCORES):
        b, ns = core // 4, core % 4
        n0 = ns * NS
        out[n0:n0 + NS, b, :] = res.results[core]["outT"].T
    return out


# revision 44
# speedup vs baseline: 1.0600x; 1.0038x over previous
"""Bass/Trainium2 kernel for DynamicMultiheadAttention (sparse_attention).

Sharding: 8 cores = (batch b in {0,1}) x (query-slice of 512 rows).
Each core computes all 8 heads for its (b, n-slice) in transposed
orientation: scores sT[m, n] with keys m on partitions, so that
  - the relative-mask bias  -sum_r c[h,r]*M_r[m,n]  is accumulated into
    score PSUM by matmuls with a scaled-identity stationary operand,
  - softmax row-sums come free from a ones-column appended to V,
  - attn @ V needs no transposes (pT tiles are directly the stationary
    operand layout).
Key padding is applied by zeroing padded key rows of V and of the
ones-column (exactly equivalent to -inf logits). The row-constant term
scale_h * sum_r w[h,r] = scale_h cancels in softmax and is dropped; the
k-projection bias is softmax-invariant and dropped; the v bias folds
into the output bias (softmax rows sum to 1): bo' = bv @ Wo + bo.

Every TPB instruction encoding in this walrus build tolerates only ONE
semaphore wait. Two mechanisms deal with that: tiny "touch" matmuls
absorb the DMA-queue waits for PE-consumed loads (one wait each), and a
post-pass (_split_matmul_waits) moves any remaining extra waits onto
standalone single-wait EventSemaphore instructions inserted before the
offending instruction on the same engine queue.

All matmuls run as float32r (1 PE cycle/row vs 4 for fp32); projection
inputs x/W are bf16 (halves the load bytes; the score/mask/attention
path stays f32r). Measured end-to-end Frobenius relative error vs the
fp32 reference: ~5e-3.
"""

import numpy as np

N, B, D = 2048, 2, 512
H, R = 8, 3
C = D // H          # 64
NS = N // 4         # 512 query rows per core
NCORES = 8
MT = N // 128       # 16 key tiles

_cache = {}


def _build_program(reps=1):
    import concourse.bass as bass
    import concourse.mybir as mybir
    import concourse.tile as tile
    from concourse.tile import add_dep_helper
    from contextlib import ExitStack

    f32 = mybir.dt.float32
    f32r = mybir.dt.float32r
    bf16 = mybir.dt.bfloat16
    u8 = mybir.dt.uint8
    AFT = mybir.ActivationFunctionType
    ALU = mybir.AluOpType

    nc = bass.Bass()

    xtq = nc.declare_dram_parameter("xtq", [D, NS], bf16, isOutput=False)
    xtk = nc.declare_dram_parameter("xtk", [D, N], bf16, isOutput=False)
    xtv = nc.declare_dram_parameter("xtv", [D, N], bf16, isOutput=False)
    masksT = nc.declare_dram_parameter("masksT", [R, N, NS], u8, isOutput=False)
    wq = nc.declare_dram_parameter("wq", [D, D], bf16, isOutput=False)
    wk = nc.declare_dram_parameter("wk", [D, D], bf16, isOutput=False)
    wv = nc.declare_dram_parameter("wv", [D, D], bf16, isOutput=False)
    wo = nc.declare_dram_parameter("wo", [D, D], bf16, isOutput=False)
    idents = nc.declare_dram_parameter("idents", [128, H * R * 128], f32r, isOutput=False)
    bq2 = nc.declare_dram_parameter("bq2", [128, 4], f32, isOutput=False)
    bo2 = nc.declare_dram_parameter("bo2", [128, 4], f32, isOutput=False)
    pad = nc.declare_dram_parameter("pad", [128, MT], f32, isOutput=False)
    pad8 = nc.declare_dram_parameter("pad8", [128, MT, H], f32, isOutput=False)
    outT = nc.declare_dram_parameter("outT", [D, NS], f32, isOutput=True)

    with tile.TileContext(nc) as tc, ExitStack() as ctx:
        # fp32 matmul is 4 cycles/row on PE; fp32r streams at 1 --
        # all matmul-operand tiles are float32r
        mm = nc.tensor.matmul

        for _rep in range(reps):
            _run_once(nc, tc, ctx, mm, tile, mybir, f32, f32r, bf16, u8,
                      AFT, ALU, xtq, xtk, xtv, masksT, wq, wk, wv, wo,
                      idents, bq2, bo2, pad, pad8, outT)

    _split_matmul_waits(nc, mybir)
    return nc


def _run_once(nc, tc, ctx, mm, tile, mybir, f32, f32r, bf16, u8, AFT, ALU,
              xtq, xtk, xtv, masksT, wq, wk, wv, wo, idents, bq2, bo2,
              pad, pad8, outT):
    from concourse.tile import add_dep_helper
    from contextlib import ExitStack
    with ExitStack() as ctx:
        const_pool = ctx.enter_context(tc.tile_pool(name="const", bufs=1))
        persist = ctx.enter_context(tc.tile_pool(name="persist", bufs=1))

        loads = []
        id_sb = const_pool.tile([128, H * R, 128], f32r)
        nc.gpsimd.dma_start(id_sb[:], idents.rearrange("p (i m) -> p i m", m=128))
        bq_sb = const_pool.tile([128, 4], f32)
        loads.append(nc.sync.dma_start(bq_sb[:], bq2[:]))
        bo_sb = const_pool.tile([128, 4], f32)
        loads.append(nc.sync.dma_start(bo_sb[:], bo2[:]))
        pad_sb = const_pool.tile([128, MT], f32)
        loads.append(nc.sync.dma_start(pad_sb[:], pad[:]))
        pad8_sb = const_pool.tile([128, MT, H], f32)
        loads.append(nc.sync.dma_start(pad8_sb[:], pad8[:]))
        ones_sb = const_pool.tile([1, 64], f32)
        loads.append(nc.vector.memset(ones_sb[:], 1.0))
        wo_sb = persist.tile([128, 4, D], bf16)
        for c in range(4):
            loads.append(nc.sync.dma_start(wo_sb[:, c, :], wo[c * 128:(c + 1) * 128, :]))

        kT_sb = persist.tile([128, 4, N], f32r)
        qT_sb = persist.tile([128, 4, NS], f32r)
        v_sb = persist.tile([128, MT, H, C + 1], f32r)
        OT_sb = persist.tile([128, 4, NS], bf16)
        outT_sb = persist.tile([128, 4, NS], f32)

        # ---- Phase A: projections ----
        with tc.tile_pool(name="xw", bufs=1) as xw_pool, \
             tc.tile_pool(name="psA", bufs=3, space="PSUM") as psA:
            wq_sb = xw_pool.tile([128, 4, D], bf16, tag="w")
            wk_sb = xw_pool.tile([128, 4, D], bf16, tag="w2")
            wv_sb = xw_pool.tile([128, 4, D], bf16, tag="w3")
            xtq_sb = xw_pool.tile([128, 4, NS], bf16, tag="xq")
            xtk_sb = xw_pool.tile([128, 4, N], bf16, tag="xk")
            xtv_sb = xw_pool.tile([128, 4, N], bf16, tag="xv")
            for c in range(4):
                sl = slice(c * 128, (c + 1) * 128)
                loads.append(nc.sync.dma_start(wq_sb[:, c, :], wq[sl, :]))
                loads.append(nc.sync.dma_start(xtq_sb[:, c, :], xtq[sl, :]))
            for c in range(4):
                sl = slice(c * 128, (c + 1) * 128)
                loads.append(nc.sync.dma_start(wk_sb[:, c, :], wk[sl, :]))
                loads.append(nc.sync.dma_start(xtk_sb[:, c, :], xtk[sl, :]))
            for c in range(4):
                sl = slice(c * 128, (c + 1) * 128)
                loads.append(nc.sync.dma_start(wv_sb[:, c, :], wv[sl, :]))
                loads.append(nc.sync.dma_start(xtv_sb[:, c, :], xtv[sl, :]))

            vones = []
            for mt in range(MT):
                vones.append(nc.vector.tensor_copy(
                    v_sb[:, mt, :, C : C + 1],
                    pad8_sb[:, mt, :].rearrange("p (h o) -> p h o", o=1)))

            projc = []
            # qT[dh, n] = (Wq/8).T @ xT_q  (+ bq/8 per-partition)
            for j in range(4):
                ps = psA.tile([128, NS], f32, tag="psA")
                for kc in range(4):
                    mm(ps[:], wq_sb[:, kc, j * 128:(j + 1) * 128],
                       xtq_sb[:, kc, :], start=(kc == 0), stop=(kc == 3))
                projc.append(nc.scalar.activation(
                    qT_sb[:, j, :], ps[:], AFT.Identity,
                    bias=bq_sb[:, j:j + 1]))

            # kT[dh, m] = Wk.T @ xT_k   (k bias is softmax-invariant: dropped)
            for mb in range(4):
                for j in range(4):
                    ps = psA.tile([128, NS], f32, tag="psA")
                    for kc in range(4):
                        mm(ps[:], wk_sb[:, kc, j * 128:(j + 1) * 128],
                           xtk_sb[:, kc, mb * 512:(mb + 1) * 512],
                           start=(kc == 0), stop=(kc == 3))
                    if j < 2:
                        projc.append(nc.scalar.copy(
                            kT_sb[:, j, mb * 512:(mb + 1) * 512], ps[:]))
                    else:
                        projc.append(nc.vector.tensor_copy(
                            kT_sb[:, j, mb * 512:(mb + 1) * 512], ps[:]))

            # v[m, c] = xT_v.T @ Wv, padded key rows zeroed (scale by pad01)
            for mt in range(MT):
                ps = psA.tile([128, D], f32, tag="psA")
                for kc in range(4):
                    mm(ps[:], xtv_sb[:, kc, mt * 128:(mt + 1) * 128],
                       wv_sb[:, kc, :], start=(kc == 0), stop=(kc == 3))
                projc.append(nc.vector.tensor_scalar(
                    v_sb[:, mt, :, 0:C],
                    ps[:].rearrange("p (h c) -> p h c", h=H),
                    pad_sb[:, mt:mt + 1], None, ALU.mult))

        # ---- Phase B: attention, two passes of 4 heads ----
        with tc.tile_pool(name="mu8", bufs=2) as mu8_pool, \
             tc.tile_pool(name="mc", bufs=2) as mc_pool, \
             tc.tile_pool(name="pT", bufs=3) as pT_pool, \
             tc.tile_pool(name="small", bufs=8) as small_pool, \
             tc.tile_pool(name="psO", bufs=4, space="PSUM") as psO, \
             tc.tile_pool(name="psB", bufs=2, space="PSUM") as psB, \
             tc.tile_pool(name="psS", bufs=2, space="PSUM") as psS:
            # masks fit in SBUF -- DMA once (pass 0), reuse in pass 1
            mall = persist.tile([128, MT, R, NS], u8, name="mall")
            prev_norm = []
            for p in range(2):
                o_ps = [psO.tile([128, NS], f32, tag="psO", name=f"o_ps{p}_{i}")
                        for i in range(4)]
                for mt in range(MT):
                    if p == 0:
                        for r in range(R):
                            nc.sync.dma_start(
                                mall[:, mt, r, :],
                                masksT[r, mt * 128:(mt + 1) * 128, :])
                    mc = mc_pool.tile([128, R, NS], f32r, tag="mc")
                    cast = nc.vector.tensor_copy(mc[:], mall[:, mt, :, :])
                    for pn in prev_norm:
                        add_dep_helper(cast.ins, pn.ins, sync=False,
                                       reason="order cast after prior norm")
                    prev_norm = []
                    for i in range(4):
                        h = 4 * p + i
                        hj, ho = h // 2, (h % 2) * 64
                        s_ps = psS.tile([128, NS], f32, tag="psS")
                        mm(s_ps[:],
                           kT_sb[ho:ho + 64, hj, mt * 128:(mt + 1) * 128],
                           qT_sb[ho:ho + 64, hj, :], start=True, stop=False)
                        for r in range(R):
                            mm(s_ps[:], id_sb[:, h * R + r, :],
                               mc[:, r, :], start=False, stop=(r == 2))
                        pT = pT_pool.tile([128, NS], f32r, tag="pT")
                        nc.scalar.activation(pT[:], s_ps[:], AFT.Exp)
                        mm(o_ps[i][0:65, :], v_sb[:, mt, h, :], pT[:],
                           start=(mt == 0), stop=(mt == MT - 1))
                # normalize: OT[h-rows, n] = o[c, n] / rowsum[n]
                norm = []
                for i in range(4):
                    h = 4 * p + i
                    hj, ho = h // 2, (h % 2) * 64
                    rsb = small_pool.tile([1, NS], f32, tag="rsb")
                    nc.vector.reciprocal(rsb[:], o_ps[i][64:65, :])
                    # bcast matmul + psum consumers kept DVE-only so the
                    # matmul's deps collapse to one semaphore wait
                    b_ps = psB.tile([128, NS], f32, tag="psB")
                    mm(b_ps[0:64, :], ones_sb[0:1, :], rsb[0:1, :],
                       start=True, stop=True)
                    b_sb = small_pool.tile([64, NS], f32, tag="bsb")
                    nc.vector.tensor_copy(b_sb[:], b_ps[0:64, :])
                    norm.append(nc.vector.tensor_tensor(
                        OT_sb[ho:ho + 64, hj, :], o_ps[i][0:64, :],
                        b_sb[:], ALU.mult))
                prev_norm = norm

            # ---- Phase C: output projection ----
            for jt in range(4):
                ps = psS.tile([128, NS], f32, tag="psS")
                for g in range(4):
                    mm(ps[:], wo_sb[:, g, jt * 128:(jt + 1) * 128],
                       OT_sb[:, g, :], start=(g == 0), stop=(g == 3))
                nc.scalar.activation(outT_sb[:, jt, :], ps[:], AFT.Identity,
                                     bias=bo_sb[:, jt:jt + 1])
                nc.sync.dma_start(outT[jt * 128:(jt + 1) * 128, :],
                                  outT_sb[:, jt, :])


# every TPB instruction encoding in this walrus build tolerates only a
# single semaphore wait -- split extras regardless of opcode
_NO_SPLIT_TYPES = {"InstEventSemaphore"}


def _split_matmul_waits(nc, mybir):
    """Several engine instruction encodings tolerate only one semaphore
    wait; move extra waits onto standalone single-wait EventSemaphore
    instructions inserted right before them on the same engine queue."""
    import bass_rust

    n = 0
    for bb in nc.m.functions[0].blocks:
        insts = list(bb.instructions)
        out = []
        changed = False
        for i in insts:
            si = i.sync_info
            if (type(i).__name__ not in _NO_SPLIT_TYPES and si is not None
                    and len(si.on_wait) > 1):
                w = list(si.on_wait)
                for wx in w[:-1]:
                    ev = mybir.InstEventSemaphore(name=f"mmw_{n}_{i.name}",
                                                  ins=[], outs=[])
                    ev.engine = i.engine
                    ev.sync_info = bass_rust.SyncInfo(on_wait=[wx],
                                                      on_update=[])
                    out.append(ev)
                    n += 1
                si.on_wait = [w[-1]]
                changed = True
            out.append(i)
        if changed:
            bb.instructions = out


def _host_prep(inputs):
    x_q = np.asarray(inputs["x_q"], np.float32)
    x_k = np.asarray(inputs["x_k"], np.float32)
    x_v = np.asarray(inputs["x_v"], np.float32)
    attn_mask = np.asarray(inputs["attn_mask"]).astype(np.uint8)
    kpm = np.asarray(inputs["key_padding_mask"]).astype(bool)
    Wq = np.asarray(inputs["Wq"], np.float32)
    Wk = np.asarray(inputs["Wk"], np.float32)
    Wv = np.asarray(inputs["Wv"], np.float32)
    Wo = np.asarray(inputs["Wo"], np.float32)
    bq = np.asarray(inputs["bq"], np.float32)
    bv = np.asarray(inputs["bv"], np.float32)
    bo = np.asarray(inputs["bo"], np.float32)
    mw = np.asarray(inputs["mask_weight"], np.float64)

    # c[h,r] = softmax(mask_weight[h,:R]) * mask_weight[h,R]
    e = np.exp(mw[:, :R] - mw[:, :R].max(axis=1, keepdims=True))
    w = e / e.sum(axis=1, keepdims=True)
    c = (w * mw[:, R:R + 1]).astype(np.float32)          # [H, R]

    idents = np.zeros((H * R, 128, 128), np.float32)
    eye = np.eye(128, dtype=np.float32)
    for h in range(H):
        for r in range(R):
            idents[h * R + r] = eye * (-c[h, r])
    # partition-major so the DMA is one contiguous descriptor per row
    idents = np.ascontiguousarray(
        idents.transpose(1, 0, 2)).reshape(128, H * R * 128)

    scale = np.float32(1.0 / np.sqrt(C))
    wq_s = (Wq * scale).astype(np.float32)
    bq_s = (bq * scale).astype(np.float32)
    bo_p = (bv @ Wo + bo).astype(np.float32)

    bq2 = np.ascontiguousarray(bq_s.reshape(4, 128).T)
    bo2 = np.ascontiguousarray(bo_p.reshape(4, 128).T)

    bf = ml_dtypes.bfloat16
    common = dict(wq=wq_s.astype(bf), wk=Wk.astype(bf), wv=Wv.astype(bf),
                  wo=Wo.astype(bf), idents=idents, bq2=bq2, bo2=bo2)

    in_maps = []
    for core in range(NCORES):
        b, ns = core // 4, core % 4
        n0 = ns * NS
        pad01 = (~kpm[b]).astype(np.float32)             # [N]
        pad2 = np.ascontiguousarray(pad01.reshape(MT, 128).T)
        pad8 = np.ascontiguousarray(np.repeat(pad2[:, :, None], H, axis=2))
        m = dict(common)
        m["xtq"] = np.ascontiguousarray(x_q[n0:n0 + NS, b, :].T).astype(bf)
        m["xtk"] = np.ascontiguousarray(x_k[:, b, :].T).astype(bf)
        m["xtv"] = np.ascontiguousarray(x_v[:, b, :].T).astype(bf)
        m["masksT"] = np.ascontiguousarray(
            attn_mask[b, :, n0:n0 + NS, :].transpose(0, 2, 1))
        m["pad"] = pad2
        m["pad8"] = pad8
        in_maps.append(m)
    return in_maps


def kernel(**inputs) -> np.ndarray:
    from concourse.bass_utils import run_bass_kernel_spmd

    if "nc" not in _cache:
        _cache["nc"] = _build_program()
    nc = _cache["nc"]

    in_maps = _host_prep(inputs)
    res = run_bass_kernel_spmd(nc, in_maps, list(range(NCORES)))

    out = np.empty((N, B, D), np.float32)
    for core in range(NCORES):
        b, ns = core // 4, core % 4
        n0 = ns * NS
        out[n0:n0 + NS, b, :] = res.results[core]["outT"].T
    return out
